# revision 51
# baseline (speedup 1.0000x reference)
"""Trainium2 Bass kernel for BatchGraphAttention (GAT-style layer), v10.

Math per sample b (one NeuronCore each, B=8 across 8 cores):
  feats  = X @ kernel[h] (+ bias[h], folded in via an augmented ones row)
  a_s    = X @ W_s[h],  a_n = X @ W_n[h]
  score  = leaky_relu(a_s[j] + a_n[i], 0.2)
  w      = softmax_j(score masked by A);  out = relu(w @ feats + bias)

Row-normalized identity (divide row i by exp(a_n[i]), cancels in softmax):
  p[j,i] = A^T[j,i] * max(es1[j], es2[j] * g[i])
  es1 = exp(a_s), es2 = exp(0.2 a_s), g = exp(-0.8 a_n)

Every (h, jc) tile uses this SAME identity, so the per-tile q can be
produced on EITHER engine (softmax rows stay consistent within a head):
  - DVE form:    q = (g_bc * es2[j]) max es1[j]   one tensor_scalar (4x)
  - scalar form: t = Relu(es2[j]*g_bc - es1[j]);  q = Identity(t + es1[j])
                 two ACTs == max(es2*g, es1) exactly.
The mask multiply p = q * A^T always runs on the DVE (2x tensor_tensor).
The form mix (S_SLOTS) balances DVE vs scalar load; measured optimum is
~16/64 scalar tiles.

Engine budget (measured): DVE ~140us (TT 78 + TS 35 + casts/misc), scalar
~125us (pairs + drains + finish), PE ~150us busy (256 agg matmuls at
mixed HAM clocks + transposes + prep).  All three are co-saturated; the
kernel runs at their common envelope.  gpsimd elementwise offload was
measured CATASTROPHIC (3 tiles: 165->262us, SBUF contention stalls the
DVE ~5x) — do not use gpsimd for anything but DMA issues and memsets.
NOTE: the device throttles ~20% under sustained back-to-back benching
(TT 1215->1470ns); compare timings only after a ~150s cool-down.

Startup: a_n rows ("beats") -> exp -> PE ones-outer-product broadcast;
head-0 chain pre-main, later heads paced into early slots; fa matmuls
produce feats + a_s columns; es quarters batched (two ACTs each).
Aggregation keeps feats|ones stationary, streams p; out^T accumulates
per head in PSUM, is PE-transposed back, normalized (relu(num*recip)),
and stored per 128-row chunk.
"""

import sys

sys.path.insert(0, "/opt/trn_rl_repo")

import ml_dtypes  # noqa: E402
import numpy as np  # noqa: E402

import concourse.bacc as bacc  # noqa: E402
import concourse.mybir as mybir  # noqa: E402
from concourse import bass_utils, tile  # noqa: E402

B, N, F, H, FO = 8, 2048, 64, 4, 32
NT = N // 128  # 16 chunks of 128 nodes
FE = FO + 1  # feats plus the ones column for the denominator
FA = F + 1  # contraction depth incl. the bias ones-row
KWC = H * FO + 2 * H  # 136: packed kernel cols + W_s cols + W_n cols
WS0 = H * FO  # W_s columns at 128..131
WN0 = H * FO + H  # W_n columns at 132..135
dt = mybir.dt
OP = mybir.AluOpType
ACT = mybir.ActivationFunctionType

# tuning knobs
LA = 3  # matmul lookahead (tiles): cool-verified sweep 2=157.7us, 3=156.2us, 4=157.8us
# scalar-assist slots per head (jc indices whose q comes from the scalar
# engine), balancing DVE (TS+TT vs TT-only) against scalar (2 ACTs/slot).
S_SLOTS = (
    (7, 10, 13),            # h0: late, so pairs never block h1 beat ACTs
    (1, 3, 6, 9, 12, 14),   # h1
    (1, 3, 6, 9, 12, 14),   # h2
    (1, 3, 5, 7, 9, 11),    # h3: keep the tail q-form so scalar can finish
)
PAIR_AHEAD = 3  # emit the scalar ACT pair this many slots before its TT
OT_BUFS = 1  # 1 vs 2 measured within run-to-run noise (~±4us, HAM phase)


def _build_nc():
    nc = bacc.Bacc(
        "TRN2",
        target_bir_lowering=False,
        debug=False,
        enable_asserts=False,
        num_devices=B,
    )
    XT_d = nc.dram_tensor("XT", [FA, N], dt.bfloat16, kind="ExternalInput")
    AT_d = nc.dram_tensor("AT", [N, N], dt.bfloat16, kind="ExternalInput")
    KW_d = nc.dram_tensor("KW", [FA, KWC], dt.bfloat16, kind="ExternalInput")
    I_d = nc.dram_tensor("IDENT", [128, 128], dt.float32, kind="ExternalInput")
    O_d = nc.dram_tensor("OUT", [N, H * FO], dt.bfloat16, kind="ExternalOutput")

    with tile.TileContext(nc) as tc:
        with (
            tc.tile_pool(name="const", bufs=1) as cpool,
            tc.tile_pool(name="work", bufs=2) as wpool,
            tc.tile_pool(name="misc", bufs=3, space="PSUM") as mp,
            tc.tile_pool(name="accp", bufs=1, space="PSUM") as acc_pool,
        ):
            ident = cpool.tile([128, 128], dt.float32, name="ident")
            X_Tb = cpool.tile([FA, N], dt.bfloat16, name="X_Tb")
            kwb = cpool.tile([FA, KWC], dt.bfloat16, name="kwb")
            ones_bf = cpool.tile([1, 128], dt.bfloat16, name="ones_bf")
            a_s4 = cpool.tile([128, NT * H], dt.float32, name="a_s4")
            es1 = cpool.tile([128, NT * H], dt.float32, name="es1")
            es2 = cpool.tile([128, NT * H], dt.float32, name="es2")
            es1n = cpool.tile([128, NT * H], dt.float32, name="es1n")
            feats = cpool.tile([128, H * NT * FE], dt.bfloat16, name="feats")
            g_row = {
                h: cpool.tile([1, N], dt.bfloat16, name=f"g_row{h}") for h in range(H)
            }
            g_bc = cpool.tile([128, H * N], dt.bfloat16, name="g_bc")
            out_sb = cpool.tile([128, NT * 128], dt.bfloat16, name="out_sb")
            recip = cpool.tile([128, H * NT], dt.float32, name="recip")
            at_full = cpool.tile([128, NT * N], dt.bfloat16, name="at_full")

            # ---- input DMAs: full-width A^T tiles (4KB rows) spread
            # across the sync/gpsimd/scalar queues in consumption (jc)
            # order; X first on scalar, kwb first on gpsimd.
            def at_dma(eng, jc):
                eng.dma_start(
                    at_full[:, jc * N : (jc + 1) * N],
                    AT_d.ap()[jc * 128 : (jc + 1) * 128, :],
                )

            # X as ONE issue: full 4KB rows (chunked issues degrade to
            # 1-2KB strided segments and starve behind the A^T streams)
            nc.scalar.dma_start(X_Tb[:, :], XT_d.ap())
            nc.gpsimd.dma_start(kwb[:, :], KW_d.ap())
            for k in range(12):
                at_dma((nc.sync, nc.gpsimd)[k % 2], k)
            nc.sync.dma_start(ident[:, :], I_d.ap())
            nc.gpsimd.memset(
                feats[:, :].rearrange("p (k w) -> p k w", w=FE)[:, :, FO : FO + 1],
                1.0,
            )
            nc.gpsimd.memset(ones_bf[:, :], 1.0)

            # ---- prep pieces ----
            feats4 = feats[:, :].rearrange("p (h t e) -> p h t e", h=H, t=NT)

            def emit_beat(h, c):
                # a_n row chunk for head h -> g_row[h] = exp(-0.8 a_n)
                ps_g = mp.tile([1, 512], dt.float32, tag="sm", name="ps_g")
                nc.tensor.matmul(
                    ps_g[:, :],
                    kwb[:, WN0 + h : WN0 + h + 1],
                    X_Tb[:, c * 512 : (c + 1) * 512],
                    start=True,
                    stop=True,
                )
                nc.scalar.activation(
                    g_row[h][:, c * 512 : (c + 1) * 512],
                    ps_g[:, :],
                    ACT.Exp,
                    scale=-0.8,
                )

            def emit_fa(t):
                ps_fa = mp.tile([128, KWC], dt.float32, tag="sm", name="ps_fa")
                nc.tensor.matmul(
                    ps_fa[:, :],
                    X_Tb[:, t * 128 : (t + 1) * 128],
                    kwb[:, :],
                    start=True,
                    stop=True,
                )
                nc.vector.tensor_copy(
                    a_s4[:, t * H : (t + 1) * H], ps_fa[:, WS0 : WS0 + H]
                )
                nc.vector.tensor_copy(
                    feats4[:, :, t, 0:FO],
                    ps_fa[:, 0:WS0].rearrange("p (h o) -> p h o", h=H),
                )

            def emit_bcast(h, c):
                ps_b = mp.tile([128, 512], dt.float32, tag="sm", name="ps_b")
                nc.tensor.matmul(
                    ps_b[:, :],
                    ones_bf[:, :],
                    g_row[h][:, c * 512 : (c + 1) * 512],
                    start=True,
                    stop=True,
                )
                nc.vector.tensor_copy(
                    g_bc[:, h * N + c * 512 : h * N + (c + 1) * 512], ps_b[:, :]
                )

            def emit_es(c):
                # es1/es2/es1n for the 4 tiles of quarter c, all heads at
                # once (layout is (t, h) so a quarter is contiguous).
                lo, hi = c * 4 * H, (c + 1) * 4 * H
                nc.scalar.activation(es1[:, lo:hi], a_s4[:, lo:hi], ACT.Exp)
                nc.scalar.activation(
                    es2[:, lo:hi], a_s4[:, lo:hi], ACT.Exp, scale=0.2
                )
                nc.vector.tensor_scalar_mul(es1n[:, lo:hi], es1[:, lo:hi], -1.0)

            # pre-main prep: heads 0-1 g chains, fa 0-7 (es quarters 0-1).
            # NOTE: tile emission order IS the dataflow program order — a
            # read emitted before its writer reads garbage, so every
            # paced item below must precede its first reader's slot.
            # (A slimmer pre-main with h1's chain paced into slots 0-3
            # measured 191us: the bcast matmuls stall behind head-0 pair
            # ACTs on the scalar queue and block the in-order PE queue.)
            for c in range(4):
                emit_beat(0, c)
            for t in range(4):
                emit_fa(t)
            emit_es(0)
            for c in range(4):
                emit_bcast(0, c)
            # late A^T tiles on the scalar ring: issued here (after the
            # beat ACTs above are queued) so the issue slices can't delay
            # them; adds a third HBM stream for the DMA-gated head 0.
            for k in range(12, NT):
                at_dma(nc.scalar, k)

            prep_sched = {
                0: [("fa", 4), ("fa", 5), ("beat", 1, 0), ("beat", 1, 1)],
                1: [("fa", 6), ("fa", 7), ("beat", 1, 2), ("beat", 1, 3)],
                2: [("es", 1), ("bcast", 1, 0), ("bcast", 1, 1)],
                3: [("bcast", 1, 2), ("bcast", 1, 3), ("fa", 8)],
                4: [("fa", 9), ("fa", 10)],
                5: [("fa", 11), ("es", 2)],
                6: [("fa", 12), ("fa", 13)],
                7: [("fa", 14), ("fa", 15)],
                8: [("es", 3)],
                9: [("beat", 2, 0)], 10: [("beat", 2, 1)], 11: [("beat", 2, 2)],
                12: [("beat", 2, 3)], 13: [("beat", 3, 0)], 14: [("beat", 3, 1)],
                15: [("beat", 3, 2)], 16: [("beat", 3, 3)],
                17: [("bcast", 2, 0)], 18: [("bcast", 2, 1)],
                19: [("bcast", 2, 2)], 20: [("bcast", 2, 3)],
                21: [("bcast", 3, 0)], 22: [("bcast", 3, 1)],
                23: [("bcast", 3, 2)], 24: [("bcast", 3, 3)],
            }

            def emit_prep_items(idx):
                for it in prep_sched.get(idx, ()):
                    if it[0] == "fa":
                        emit_fa(it[1])
                    elif it[0] == "es":
                        emit_es(it[1])
                    elif it[0] == "beat":
                        emit_beat(it[1], it[2])
                    else:
                        emit_bcast(it[1], it[2])

            # ---- main loop machinery ----
            def escol(buf, h, jc):
                k = jc * H + h
                return buf[:, k : k + 1]

            sq = {}  # (h, jc) -> scalar-produced q tile

            def emit_pair(h, jc):
                # scalar form: t = Relu(es2[j]*g - es1[j]); q = t + es1[j]
                u = wpool.tile([128, N], dt.bfloat16, tag="u", name="u", bufs=2)
                nc.scalar.activation(
                    u[:, :],
                    g_bc[:, h * N : (h + 1) * N],
                    ACT.Relu,
                    bias=escol(es1n, h, jc),
                    scale=escol(es2, h, jc),
                )
                q = wpool.tile([128, N], dt.bfloat16, tag="sq", name="sq", bufs=4)
                nc.scalar.activation(
                    q[:, :], u[:, :], ACT.Identity, bias=escol(es1, h, jc)
                )
                sq[(h, jc)] = q

            def emit_elem(h, jc):
                if (h, jc) in sq:
                    q = sq.pop((h, jc))
                else:
                    q = wpool.tile([128, N], dt.bfloat16, tag="q", name="q", bufs=2)
                    nc.vector.tensor_scalar(
                        q[:, :],
                        g_bc[:, h * N : (h + 1) * N],
                        escol(es2, h, jc),
                        escol(es1, h, jc),
                        OP.mult,
                        OP.max,
                    )
                p = wpool.tile([128, N], dt.bfloat16, tag="p", name="p", bufs=LA + 2)
                nc.vector.tensor_tensor(
                    p[:, :], q[:, :], at_full[:, jc * N : (jc + 1) * N], OP.mult
                )
                return p

            def emit_mm(h, jc, p):
                k = h * NT + jc
                for c in range(4):
                    nc.tensor.matmul(
                        psum_oT[:, c * 512 : (c + 1) * 512],
                        feats[:, k * FE : (k + 1) * FE],
                        p[:, c * 512 : (c + 1) * 512],
                        start=(jc == 0),
                        stop=(jc == NT - 1),
                        skip_group_check=True,
                    )

            def emit_oT_copy(h):
                oT_sb = wpool.tile(
                    [FE, N], dt.float32, tag="oT_sb", name="oT_sb", bufs=OT_BUFS
                )
                nc.scalar.copy(oT_sb[:, :], psum_oT[:, :])
                return oT_sb

            def emit_head_finish(h, oT_sb):
                for g in range(2):
                    ps_t = mp.tile([128, 8 * 64], dt.float32, tag="sm", name="ps_t")
                    for k8 in range(8):
                        ic = g * 8 + k8
                        nc.tensor.transpose(
                            ps_t[:, k8 * 64 : k8 * 64 + FE],
                            oT_sb[:, ic * 128 : (ic + 1) * 128],
                            ident[:33, :33],
                        )
                    nc.vector.reciprocal(
                        recip[:, h * NT + g * 8 : h * NT + (g + 1) * 8].rearrange(
                            "p (k w) -> p k w", w=1
                        ),
                        ps_t[:, :].rearrange("p (k w) -> p k w", w=64)[
                            :, :, FO : FO + 1
                        ],
                    )
                    for k8 in range(8):
                        ic = g * 8 + k8
                        dst = out_sb[:, ic * 128 + h * FO : ic * 128 + (h + 1) * FO]
                        rc = recip[:, h * NT + ic : h * NT + ic + 1]
                        if k8 % 2 == 1:
                            nc.vector.tensor_scalar(
                                dst,
                                ps_t[:, k8 * 64 : k8 * 64 + FO],
                                rc,
                                0.0,
                                OP.mult,
                                OP.max,
                            )
                        else:
                            nc.scalar.activation(
                                dst,
                                ps_t[:, k8 * 64 : k8 * 64 + FO],
                                ACT.Relu,
                                scale=rc,
                            )
                    if h == H - 1:
                        # one 8-chunk DMA per half instead of 16 per-ic
                        # issues (each issue slice costs ~600ns)
                        eng = (nc.sync, nc.gpsimd)[g]
                        eng.dma_start(
                            O_d.ap()[g * 1024 : (g + 1) * 1024, :].rearrange(
                                "(k p) o -> p k o", p=128
                            ),
                            out_sb[:, g * 1024 : (g + 1) * 1024].rearrange(
                                "p (k o) -> p k o", k=8
                            ),
                        )

            # global slot order with scalar-pair prefetch
            slots = [(h, jc) for h in range(H) for jc in range(NT)]
            pair_for = {}
            for idx, (h, jc) in enumerate(slots):
                if jc in S_SLOTS[h]:
                    at = max(0, idx - PAIR_AHEAD)
                    pair_for.setdefault(at, []).append((h, jc))

            pending = None
            fifo = []
            for idx, (h, jc) in enumerate(slots):
                if jc == 0:
                    psum_oT = acc_pool.tile(
                        [FE, N], dt.float32, tag="oT", name="psum_oT"
                    )
                for hp, jp in pair_for.get(idx, ()):
                    emit_pair(hp, jp)
                emit_prep_items(idx)
                fifo.append((h, jc, emit_elem(h, jc)))
                if len(fifo) > LA:
                    emit_mm(*fifo.pop(0))
                if jc == 1 and pending is not None:
                    emit_head_finish(*pending)
                    pending = None
                if jc == NT - 1:
                    while fifo:
                        emit_mm(*fifo.pop(0))
                    pending = (h, emit_oT_copy(h))
            emit_head_finish(*pending)

    nc.compile()
    return nc


_NC = None


def _get_nc():
    global _NC
    if _NC is None:
        _NC = _build_nc()
    return _NC


def _make_in_maps(inputs):
    X = np.asarray(inputs["X"], dtype=np.float32)
    A = np.asarray(inputs["A"], dtype=np.float32)
    K = np.asarray(inputs["kernel"], dtype=np.float32)
    BS = np.asarray(inputs["bias"], dtype=np.float32).reshape(H, FO)
    AS = np.asarray(inputs["attn_self"], dtype=np.float32)
    AN = np.asarray(inputs["attn_neigh"], dtype=np.float32)

    # X^T with an appended ones row (feeds the bias row of KW)
    XT = np.concatenate(
        [X.transpose(0, 2, 1), np.ones((B, 1, N), dtype=np.float32)], axis=1
    )  # [B, 65, N]
    XT = np.ascontiguousarray(XT).astype(ml_dtypes.bfloat16)

    # KW: [65, 136] = [[K packed (f,(h,o)) | W_s cols | W_n cols],
    #                  [bias flat          | 0        | 0       ]]
    Kp = K.transpose(1, 0, 2).reshape(F, H * FO)  # [64, 128]
    Ws = np.einsum("hfo,ho->hf", K, AS)  # [H, F]
    Wn = np.einsum("hfo,ho->hf", K, AN)
    KW = np.zeros((FA, KWC), dtype=np.float32)
    KW[:F, :WS0] = Kp
    KW[:F, WS0 : WS0 + H] = Ws.T
    KW[:F, WN0 : WN0 + H] = Wn.T
    KW[F, :WS0] = BS.reshape(H * FO)  # bias row (ones row of X^T picks it up)
    KW = np.ascontiguousarray(KW).astype(ml_dtypes.bfloat16)

    # A^T in bf16 (exact: A is a 0/1 mask)
    AT = np.ascontiguousarray(A.transpose(0, 2, 1)).astype(ml_dtypes.bfloat16)

    ident = np.eye(128, dtype=np.float32)
    return [{"XT": XT[b], "AT": AT[b], "KW": KW, "IDENT": ident} for b in range(B)]


def run(inputs, trace=False, tmpdir=None):
    nc = _get_nc()
    res = bass_utils.run_bass_kernel_spmd(
        nc, _make_in_maps(inputs), core_ids=list(range(B)), trace=trace, tmpdir=tmpdir
    )
    out = np.stack([r["OUT"] for r in res.results], axis=0).astype(np.float32)
    return out, res


def kernel(**inputs):
    out, _ = run(inputs, trace=False)
    return out
